# revision 27
# baseline (speedup 1.0000x reference)
"""BalanceL1Loss on 8 Trainium2 NeuronCores.

reference semantics:
    loss = |pred[:,0] - gt|
    positive_loss = sum(loss*mask) / floor(sum(mask))
    negative_count = min(floor(sum(1-mask)), 3*floor(sum(mask)))
    negative_loss  = sum(top-k of loss*(1-mask), k=negative_count) / negative_count
    return (positive_loss + negative_loss, positive_loss, negative_loss)

Because mask has ~30% positives, 3*positive_count > negative_avail, so the
top-k selects *every* nonzero negative element and the sort collapses to a
plain sum: negative_sum = sum(loss) - sum(loss*mask).  The device kernel
therefore only needs two full reductions: sum(|pred-gt|) and
sum(|pred-gt|*mask); sum(mask) is an input-derived scalar computed on the
host.  The (never-taken for the benchmark inputs) general case is handled
by an exact host-side top-k fallback.

Sharding: data-parallel on batch N=16 -> 2 images per core.

The stream is the wall, so everything stays fp8e4m3 end-to-end on device
(1 byte/elem on both the HBM and SBUF side):
  DVE   tensor_sub   d = p - g     (fp8, 1x mode)
  ACT   Abs          l = |d|       (fp8 out; rate is dtype-independent)
  PE    diag-matmul  data is gridded into 67 blocks of 127 columns (the
        last zero-padded); each block's stationary weights are the 128
        columns [mask(127) | ones(1)], the moving tensor is l's 127
        columns, all accumulated into one [128,127] PSUM region:
          psum[m<127, n] += sum_k mask[k,m'] * l[k,n']   (diag = masked sums)
          psum[127,  n] += sum_k     1      * l[k,n']    (row = plain sums)
        so ONE matmul stream yields BOTH reductions: the diagonal is
        sum(|d|*mask) split 127 ways and row 127 is sum|d| split 127 ways.
        ACT needs no fused accumulator (saving its ~0.28us accumulator
        read per chunk), and a single 65KB output DMA carries everything;
        the host sums diagonal and row in float64.
fp8 quantization of pred/gt/diff contributes ~1.9e-3 relative error
(validated host-side), well under the 2e-2 gate.

Fixed-overhead trims: Tile's end-of-kernel double all-engine barrier is
replaced by a single join+drain, the entry-block barrier and dead const
memsets are stripped, and all input DMA issues are hoisted into the entry
block so the stream starts during engine boot.
"""

import numpy as np

N_CORES = 8
N, H, W = 16, 736, 736
P = 128
PER_CORE = (N // N_CORES) * H * W        # 1,083,392
FREE = PER_CORE // P                     # 8,464 real data columns
DB = 127                                 # data cols per matmul block
NBLK = (FREE + DB - 1) // DB             # 67 blocks
PAD_FREE = NBLK * DB                     # 8,509 (45 zero-pad cols)
# blocks per DMA chunk (data cols = 127*b, mask cols = 128*b per chunk)
CHUNK_BLOCKS = [16, 24, 24, 2, 1]
assert sum(CHUNK_BLOCKS) == NBLK
NCHUNK = len(CHUNK_BLOCKS)
NEGATIVE_RATIO = 3.0

_cache = {}


def _build_nc():
    import concourse.mybir as mybir
    from concourse import bacc, tile

    # Trimmed kernel tail: Tile's stock epilogue is drain + all-engine
    # barrier + sem clear + all-engine barrier (~9.5us of EVSEM butterflies).
    # The drain (with waits on every engine's final tick) is the only part
    # needed for completion; the runtime's own NEFF postamble resets all
    # semaphores after every execution (verified across repeated runs).
    def _drain_only(self, tick_clock, wait_clock):
        from concourse.vector_clock import ScopedClock

        drain_inst = self.nc.sync.drain()
        wait_clock.add_sem_waits(
            drain_inst.ins, ScopedClock({None: tick_clock.global_clock})
        )
        popped = self.nc._tile_sem_poison_stack.pop()
        assert popped is self._sem_poison

    fp32 = mybir.dt.float32
    fp8 = mybir.dt.float8e4
    nc = bacc.Bacc("TRN2", target_bir_lowering=False, debug=False)
    # chunk c (b blocks) is a contiguous (P, 2*127b + 128b) row-major fp8
    # region laid out [pred | gt | mask'] with mask' = per block
    # [mask(127) | 1.0]
    total_cols = sum(2 * DB * b + 128 * b for b in CHUNK_BLOCKS)
    pk_d = nc.dram_tensor("packed_s", (P * total_cols,), fp8,
                          kind="ExternalInput").ap()
    out_d = nc.dram_tensor("acc_out", (P, DB), fp32, kind="ExternalOutput").ap()

    tc_ctx = tile.TileContext(nc)
    tc_ctx._drain_and_barrier = _drain_only.__get__(tc_ctx)
    with tc_ctx as tc:
        with (
            tc.tile_pool(name="io", bufs=1) as io_pool,
            tc.tile_pool(name="work", bufs=3) as w_pool,
            tc.tile_pool(name="acc", bufs=1) as acc_pool,
            tc.tile_pool(name="ps", bufs=1, space="PSUM") as ps_pool,
        ):
            psum_sb = acc_pool.tile([P, DB], fp32)   # DVE psum copy
            psum = ps_pool.tile([P, DB], fp32)
            # explicit activation bias; the implicit bias=0.0 would read a
            # const tile whose memset lives in the (stripped) entry block
            zero_h = acc_pool.tile([P, 1], fp8)
            nc.vector.memset(zero_h[:], 0.0)

            ins = []
            base = 0
            for c, b in enumerate(CHUNK_BLOCKS):
                cols = 2 * DB * b + 128 * b
                t = io_pool.tile([P, cols], fp8, tag=f"in{c}", name="t")
                src = pk_d[base:base + P * cols].rearrange("(p f) -> p f", p=P)
                nc.sync.dma_start(t[:], src)
                base += P * cols
                ins.append(t)

            # DoubleRow fp8 matmuls: one instruction covers TWO adjacent
            # blocks (lhsT = both 128-col mask sections concatenated, rhs =
            # both 127-col l spans), halving the PE instruction count and
            # its per-row stream cycles; chunks have even block counts so
            # pairs never straddle a chunk, and the single leftover block
            # runs as a normal matmul carrying the stop flag.
            n_sb = sum(b // 2 for b in CHUNK_BLOCKS)
            mm_idx = 0
            for c, b in enumerate(CHUNK_BLOCKS):
                t = ins[c]
                dd = DB * b
                d = w_pool.tile([P, dd], fp8, tag="d", bufs=3, name="d")
                l = w_pool.tile([P, dd], fp8, tag="l", bufs=3, name="l")
                nc.vector.tensor_sub(d[:], t[:, 0:dd], t[:, dd:2 * dd])
                nc.scalar.activation(
                    l[:], d[:], mybir.ActivationFunctionType.Abs,
                    bias=zero_h[:, 0:1],
                )
                for j in range(b // 2):
                    w2 = t[:, 2 * dd + 256 * j:2 * dd + 256 * (j + 1)]
                    r2 = l[:, 2 * DB * j:2 * DB * (j + 1)]
                    nc.tensor.matmul(
                        psum[:, 0:DB],
                        w2.rearrange("p (two m) -> p two m", two=2),
                        r2.rearrange("p (two n) -> p two n", two=2),
                        start=(mm_idx == 0),
                        stop=False,
                        perf_mode=mybir.MatmulPerfMode.DoubleRow,
                    )
                    mm_idx += 1
                if b % 2:
                    assert c == NCHUNK - 1 and b == 1
                    nc.tensor.matmul(
                        psum[:, 0:DB],
                        t[:, 2 * dd:2 * dd + 128],
                        l[:, 0:DB],
                        start=False,
                        stop=True,
                    )

            # single 65KB result: diag = sum(|d|*mask), row 127 = sum|d|
            nc.vector.tensor_copy(psum_sb[:], psum[:])
            nc.sync.dma_start(out_d[:], psum_sb[:])
    nc.compile()

    # Slim the entry block: drop the dead const-tile memsets and the entry
    # all-engine barrier (drain + gather/release event sems).  Every
    # cross-engine dependency in the kernel body is sem-based, and the
    # runtime zeroes all semaphores between executions, so the engines can
    # branch straight into the kernel body after their own boot.
    blocks = nc.m.functions[0].blocks
    main_b = blocks[0]
    drop = {"InstMemset", "InstDrain", "InstEventSemaphore"}
    keep = [i for i in main_b.instructions if type(i).__name__ not in drop]
    del main_b.instructions[:]
    for i in keep:
        main_b.instructions.append(i)

    # hoist all wait-free input DMA issues into the entry block so the
    # stream starts during engine boot
    tile_b = blocks[1]
    movable = [
        i for i in list(tile_b.instructions)
        if type(i).__name__ == "InstDMACopy"
        and i.engine == mybir.EngineType.SP
        and not (i.sync_info and i.sync_info.on_wait)
    ]
    kept = [i for i in tile_b.instructions if i not in movable]
    del tile_b.instructions[:]
    for i in kept:
        tile_b.instructions.append(i)
    for pos, i in enumerate(movable):
        main_b.instructions.insert(1 + pos, i)
    return nc


def _pack(pred_r, gt_r, mask_r):
    """(P,FREE) x3 fp32 -> flat fp8: per chunk [pred | gt | mask'] where
    the data is zero-padded to 127*NBLK columns and mask' carries a 1.0
    column after every 127 mask columns."""
    import ml_dtypes

    pad = PAD_FREE - FREE
    z = np.zeros((P, pad), np.float32)
    pred_p = np.concatenate([pred_r, z], axis=1)
    gt_p = np.concatenate([gt_r, z], axis=1)
    mask_p = np.concatenate([mask_r, z], axis=1)
    ones = np.ones((P, 1), np.float32)

    parts = []
    blk = 0
    for b in CHUNK_BLOCKS:
        sl = slice(blk * DB, (blk + b) * DB)
        mcols = []
        for j in range(blk, blk + b):
            mcols.append(mask_p[:, j * DB:(j + 1) * DB])
            mcols.append(ones)
        blk += b
        parts.append(np.concatenate(
            [pred_p[:, sl], gt_p[:, sl]] + mcols,
            axis=1).astype(ml_dtypes.float8_e4m3).ravel())
    return np.ascontiguousarray(np.concatenate(parts))


def _run_device(pred, gt, mask, **spmd_kwargs):
    """Returns (sum_l, sum_p, sum_m, BassKernelResults)."""
    from concourse.bass_utils import run_bass_kernel_spmd

    if "nc" not in _cache:
        _cache["nc"] = _build_nc()
    nc = _cache["nc"]

    per = N // N_CORES
    pred_flat = np.asarray(pred, np.float32).reshape(N, H * W)
    gt_flat = np.asarray(gt, np.float32).reshape(N, H * W)
    mask_flat = np.asarray(mask, np.float32).reshape(N, H * W)

    in_maps = []
    for i in range(N_CORES):
        s = slice(i * per, (i + 1) * per)
        in_maps.append({"packed_s": _pack(pred_flat[s].reshape(P, FREE),
                                          gt_flat[s].reshape(P, FREE),
                                          mask_flat[s].reshape(P, FREE))})
    res = run_bass_kernel_spmd(nc, in_maps, list(range(N_CORES)), **spmd_kwargs)

    sum_l = sum_p = 0.0
    for o in res.results:
        a = np.asarray(o["acc_out"], np.float64)
        sum_p += np.trace(a[0:DB, 0:DB])
        sum_l += a[DB, :].sum()
    # mask sum is an input-derived scalar; exact in f64 (mask is 0/1)
    sum_m = float(mask_flat.sum(dtype=np.float64))
    return sum_l, sum_p, sum_m, res


def kernel(pred, gt, mask, **spmd_kwargs):
    sum_l, sum_p, sum_m, _ = _run_device(pred, gt, mask, **spmd_kwargs)

    total_elems = float(N * H * W)
    positive_count = np.floor(sum_m)
    negative_avail = total_elems - positive_count
    negative_count = min(negative_avail, positive_count * NEGATIVE_RATIO)

    if negative_count >= negative_avail:
        # top-k covers every nonzero negative -> plain sum
        negative_sum = sum_l - sum_p
    else:
        # exact host fallback (not hit for the benchmark distribution)
        l = np.abs(
            np.asarray(pred, np.float64).reshape(N, H * W)
            - np.asarray(gt, np.float64).reshape(N, H * W)
        )
        neg = (l * (1.0 - np.asarray(mask, np.float64).reshape(N, H * W))).ravel()
        k = int(negative_count)
        negative_sum = float(np.partition(neg, -k)[-k:].sum()) if k > 0 else 0.0

    with np.errstate(divide="ignore", invalid="ignore"):
        positive_loss = sum_p / positive_count
        negative_loss = negative_sum / negative_count
        total = positive_loss + negative_loss
    return (np.float32(total), np.float32(positive_loss), np.float32(negative_loss))


# revision 29
# speedup vs baseline: 1.1551x; 1.1551x over previous
"""BalanceL1Loss on 8 Trainium2 NeuronCores.

reference semantics:
    loss = |pred[:,0] - gt|
    positive_loss = sum(loss*mask) / floor(sum(mask))
    negative_count = min(floor(sum(1-mask)), 3*floor(sum(mask)))
    negative_loss  = sum(top-k of loss*(1-mask), k=negative_count) / negative_count
    return (positive_loss + negative_loss, positive_loss, negative_loss)

Because mask has ~30% positives, 3*positive_count > negative_avail, so the
top-k selects *every* nonzero negative element and the sort collapses to a
plain sum: negative_sum = sum(loss) - sum(loss*mask).  The device kernel
therefore only needs two full reductions: sum(|pred-gt|) and
sum(|pred-gt|*mask); sum(mask) is an input-derived scalar computed on the
host.  The (never-taken for the benchmark inputs) general case is handled
by an exact host-side top-k fallback.

Sharding: data-parallel on batch N=16 -> 2 images per core.

The stream is the wall, so everything stays fp8e4m3 end-to-end on device
(1 byte/elem on both the HBM and SBUF side):
  DVE   tensor_sub   d = p - g     (fp8, 1x mode)
  ACT   Abs          l = |d|       (fp8 out; rate is dtype-independent)
  PE    diag-matmul  data is gridded into 67 blocks of 127 columns (the
        last zero-padded); each block's stationary weights are the 128
        columns [mask(127) | ones(1)], the moving tensor is l's 127
        columns, all accumulated into one [128,127] PSUM region:
          psum[m<127, n] += sum_k mask[k,m'] * l[k,n']   (diag = masked sums)
          psum[127,  n] += sum_k     1      * l[k,n']    (row = plain sums)
        so ONE matmul stream yields BOTH reductions: the diagonal is
        sum(|d|*mask) split 127 ways and row 127 is sum|d| split 127 ways.
        ACT needs no fused accumulator (saving its ~0.28us accumulator
        read per chunk), and a single 65KB output DMA carries everything;
        the host sums diagonal and row in float64.
fp8 quantization of pred/gt/diff contributes ~1.9e-3 relative error
(validated host-side), well under the 2e-2 gate.

Fixed-overhead trims: Tile's end-of-kernel double all-engine barrier is
replaced by a single join+drain, the entry-block barrier and dead const
memsets are stripped, and all input DMA issues are hoisted into the entry
block so the stream starts during engine boot.
"""

import numpy as np

N_CORES = 8
N, H, W = 16, 736, 736
P = 128
PER_CORE = (N // N_CORES) * H * W        # 1,083,392
FREE = PER_CORE // P                     # 8,464 real data columns
DB = 127                                 # data cols per matmul block
NBLK = (FREE + DB - 1) // DB             # 67 blocks
PAD_FREE = NBLK * DB                     # 8,509 (45 zero-pad cols)
# blocks per DMA chunk (data cols = 127*b, mask cols = 128*b per chunk)
CHUNK_BLOCKS = [8, 12, 12, 12, 12, 10, 1]
assert sum(CHUNK_BLOCKS) == NBLK
NCHUNK = len(CHUNK_BLOCKS)
NEGATIVE_RATIO = 3.0

_cache = {}


def _build_nc():
    import concourse.mybir as mybir
    from concourse import bacc, tile

    # Trimmed kernel tail: Tile's stock epilogue is drain + all-engine
    # barrier + sem clear + all-engine barrier (~9.5us of EVSEM butterflies).
    # The drain (with waits on every engine's final tick) is the only part
    # needed for completion; the runtime's own NEFF postamble resets all
    # semaphores after every execution (verified across repeated runs).
    def _drain_only(self, tick_clock, wait_clock):
        from concourse.vector_clock import ScopedClock

        drain_inst = self.nc.sync.drain()
        wait_clock.add_sem_waits(
            drain_inst.ins, ScopedClock({None: tick_clock.global_clock})
        )
        popped = self.nc._tile_sem_poison_stack.pop()
        assert popped is self._sem_poison

    fp32 = mybir.dt.float32
    fp8 = mybir.dt.float8e4
    nc = bacc.Bacc("TRN2", target_bir_lowering=False, debug=False)
    # chunk c (b blocks) is a contiguous (P, 2*127b + 128b) row-major fp8
    # region laid out [pred | gt | mask'] with mask' = per block
    # [mask(127) | 1.0]
    total_cols = sum(2 * DB * b + 128 * b for b in CHUNK_BLOCKS)
    pk_d = nc.dram_tensor("packed_s", (P * total_cols,), fp8,
                          kind="ExternalInput").ap()
    out_d = nc.dram_tensor("acc_out", (P, DB), fp32, kind="ExternalOutput").ap()

    tc_ctx = tile.TileContext(nc)
    tc_ctx._drain_and_barrier = _drain_only.__get__(tc_ctx)
    with tc_ctx as tc:
        with (
            tc.tile_pool(name="io", bufs=1) as io_pool,
            tc.tile_pool(name="work", bufs=3) as w_pool,
            tc.tile_pool(name="acc", bufs=1) as acc_pool,
            tc.tile_pool(name="ps", bufs=1, space="PSUM") as ps_pool,
        ):
            psum_sb = acc_pool.tile([P, DB], fp32)   # DVE psum copy
            psum = ps_pool.tile([P, DB], fp32)

            ins = []
            base = 0
            for c, b in enumerate(CHUNK_BLOCKS):
                cols = 2 * DB * b + 128 * b
                t = io_pool.tile([P, cols], fp8, tag=f"in{c}", name="t")
                src = pk_d[base:base + P * cols].rearrange("(p f) -> p f", p=P)
                nc.sync.dma_start(t[:], src)
                base += P * cols
                ins.append(t)

            # The serial sub->abs chain paces the kernel once the stream
            # runs fast, so: GPSIMD takes two chunks' subs off DVE, and
            # fp8 abs collapses to a sign-bit clear -- the d tile bitcast
            # to uint32/16/8 and ANDed with 0x7f per byte on DVE
            # (tensor_scalar single-src runs at 2x there, 4 elems/word),
            # making abs nearly free and freeing ACT entirely.
            SUB_GP = {3, 4}            # chunks whose sub runs on GPSIMD

            def emit_abs(dst, srcv, dd):
                if dd % 4 == 0:
                    w, m = mybir.dt.uint32, 0x7F7F7F7F
                elif dd % 2 == 0:
                    w, m = mybir.dt.uint16, 0x7F7F
                else:
                    w, m = mybir.dt.uint8, 0x7F
                nc.vector.tensor_scalar(
                    dst.bitcast(w), srcv.bitcast(w), m, None,
                    mybir.AluOpType.bitwise_and)

            mm_idx = 0
            for c, b in enumerate(CHUNK_BLOCKS):
                t = ins[c]
                dd = DB * b
                d = w_pool.tile([P, dd], fp8, tag="d", bufs=3, name="d")
                l = w_pool.tile([P, dd], fp8, tag="l", bufs=3, name="l")
                if c in SUB_GP:
                    nc.gpsimd.tensor_sub(d[:], t[:, 0:dd], t[:, dd:2 * dd])
                else:
                    nc.vector.tensor_sub(d[:], t[:, 0:dd], t[:, dd:2 * dd])
                emit_abs(l[:], d[:], dd)
                for j in range(b // 2):
                    w2 = t[:, 2 * dd + 256 * j:2 * dd + 256 * (j + 1)]
                    r2 = l[:, 2 * DB * j:2 * DB * (j + 1)]
                    nc.tensor.matmul(
                        psum[:, 0:DB],
                        w2.rearrange("p (two m) -> p two m", two=2),
                        r2.rearrange("p (two n) -> p two n", two=2),
                        start=(mm_idx == 0),
                        stop=False,
                        perf_mode=mybir.MatmulPerfMode.DoubleRow,
                    )
                    mm_idx += 1
                if b % 2:
                    assert c == NCHUNK - 1 and b == 1
                    nc.tensor.matmul(
                        psum[:, 0:DB],
                        t[:, 2 * dd:2 * dd + 128],
                        l[:, 0:DB],
                        start=False,
                        stop=True,
                    )

            # single 65KB result: diag = sum(|d|*mask), row 127 = sum|d|
            nc.vector.tensor_copy(psum_sb[:], psum[:])
            nc.sync.dma_start(out_d[:], psum_sb[:])
    nc.compile()

    # Slim the entry block: drop the dead const-tile memsets and the entry
    # all-engine barrier (drain + gather/release event sems).  Every
    # cross-engine dependency in the kernel body is sem-based, and the
    # runtime zeroes all semaphores between executions, so the engines can
    # branch straight into the kernel body after their own boot.
    blocks = nc.m.functions[0].blocks
    main_b = blocks[0]
    drop = {"InstMemset", "InstDrain", "InstEventSemaphore"}
    keep = [i for i in main_b.instructions if type(i).__name__ not in drop]
    del main_b.instructions[:]
    for i in keep:
        main_b.instructions.append(i)

    # hoist all wait-free input DMA issues into the entry block so the
    # stream starts during engine boot
    tile_b = blocks[1]
    movable = [
        i for i in list(tile_b.instructions)
        if type(i).__name__ == "InstDMACopy"
        and i.engine == mybir.EngineType.SP
        and not (i.sync_info and i.sync_info.on_wait)
    ]
    kept = [i for i in tile_b.instructions if i not in movable]
    del tile_b.instructions[:]
    for i in kept:
        tile_b.instructions.append(i)
    for pos, i in enumerate(movable):
        main_b.instructions.insert(1 + pos, i)
    return nc


def _pack(pred_r, gt_r, mask_r):
    """(P,FREE) x3 fp32 -> flat fp8: per chunk [pred | gt | mask'] where
    the data is zero-padded to 127*NBLK columns and mask' carries a 1.0
    column after every 127 mask columns."""
    import ml_dtypes

    pad = PAD_FREE - FREE
    z = np.zeros((P, pad), np.float32)
    pred_p = np.concatenate([pred_r, z], axis=1)
    gt_p = np.concatenate([gt_r, z], axis=1)
    mask_p = np.concatenate([mask_r, z], axis=1)
    ones = np.ones((P, 1), np.float32)

    parts = []
    blk = 0
    for b in CHUNK_BLOCKS:
        sl = slice(blk * DB, (blk + b) * DB)
        mcols = []
        for j in range(blk, blk + b):
            mcols.append(mask_p[:, j * DB:(j + 1) * DB])
            mcols.append(ones)
        blk += b
        parts.append(np.concatenate(
            [pred_p[:, sl], gt_p[:, sl]] + mcols,
            axis=1).astype(ml_dtypes.float8_e4m3).ravel())
    return np.ascontiguousarray(np.concatenate(parts))


def _run_device(pred, gt, mask, **spmd_kwargs):
    """Returns (sum_l, sum_p, sum_m, BassKernelResults)."""
    from concourse.bass_utils import run_bass_kernel_spmd

    if "nc" not in _cache:
        _cache["nc"] = _build_nc()
    nc = _cache["nc"]

    per = N // N_CORES
    pred_flat = np.asarray(pred, np.float32).reshape(N, H * W)
    gt_flat = np.asarray(gt, np.float32).reshape(N, H * W)
    mask_flat = np.asarray(mask, np.float32).reshape(N, H * W)

    in_maps = []
    for i in range(N_CORES):
        s = slice(i * per, (i + 1) * per)
        in_maps.append({"packed_s": _pack(pred_flat[s].reshape(P, FREE),
                                          gt_flat[s].reshape(P, FREE),
                                          mask_flat[s].reshape(P, FREE))})
    res = run_bass_kernel_spmd(nc, in_maps, list(range(N_CORES)), **spmd_kwargs)

    sum_l = sum_p = 0.0
    for o in res.results:
        a = np.asarray(o["acc_out"], np.float64)
        sum_p += np.trace(a[0:DB, 0:DB])
        sum_l += a[DB, :].sum()
    # mask sum is an input-derived scalar; exact in f64 (mask is 0/1)
    sum_m = float(mask_flat.sum(dtype=np.float64))
    return sum_l, sum_p, sum_m, res


def kernel(pred, gt, mask, **spmd_kwargs):
    sum_l, sum_p, sum_m, _ = _run_device(pred, gt, mask, **spmd_kwargs)

    total_elems = float(N * H * W)
    positive_count = np.floor(sum_m)
    negative_avail = total_elems - positive_count
    negative_count = min(negative_avail, positive_count * NEGATIVE_RATIO)

    if negative_count >= negative_avail:
        # top-k covers every nonzero negative -> plain sum
        negative_sum = sum_l - sum_p
    else:
        # exact host fallback (not hit for the benchmark distribution)
        l = np.abs(
            np.asarray(pred, np.float64).reshape(N, H * W)
            - np.asarray(gt, np.float64).reshape(N, H * W)
        )
        neg = (l * (1.0 - np.asarray(mask, np.float64).reshape(N, H * W))).ravel()
        k = int(negative_count)
        negative_sum = float(np.partition(neg, -k)[-k:].sum()) if k > 0 else 0.0

    with np.errstate(divide="ignore", invalid="ignore"):
        positive_loss = sum_p / positive_count
        negative_loss = negative_sum / negative_count
        total = positive_loss + negative_loss
    return (np.float32(total), np.float32(positive_loss), np.float32(negative_loss))
